# revision 16
# baseline (speedup 1.0000x reference)
"""Trainium2 Bass kernel for the FNO-style FourierLayer.

  x: [8, 512, 512, 32] f32 -> rfft2 over (h, w) -> keep 32x32 modes ->
  per-mode (C x C) channel mix with W[32, 32, 32, 32] -> zero-pad -> irfft2.

Strategy: data-parallel over batch, one sample per NeuronCore (8 cores).
Only 32 of 512 frequencies survive, so instead of an FFT each core runs a
chain of small dense real matmuls against DFT basis matrices (bf16 operands,
fp32 PSUM accumulation):

  A:   P = F^T X         contract h       -> [kxri 64, (w c)]
  T1:  PE transposes     w onto partitions
  B:   raw = G^T PT      contract w       -> [kyri 64, (c kxri)] psum accum
  Tc:  PE transposes     c onto partitions -> rawT [32, (kx rx kyri)]
  CMB: complex re/im combine (DVE, column-wise) -> lowT [32, (kx ry ky)]
  C:   per-mode matmuls  OL = W^T low     -> [d 32, (kx ky ri)]
  Tpc: PE transposes     kyri onto partitions -> OLT [64, (kx d)]
  D:   u = Dab^T OLT     contract kyri    -> u_wc [w 128, (ab kx d)]
  T2:  PE transposes     (ab kx) onto partitions -> uT [64, (w d)]
  E:   out = Einv^T uT   contract (ab kx) -> [h 128, (w d)] -> DMA out

DFT matrices are built on host from np.fft basis responses (this captures
the irfft Im(DC)-drop convention exactly). x, W and the matrices are cast
to bf16 on host and the output is returned as bf16 and upcast on host,
which halves DMA traffic in both directions.
"""
import numpy as np
import ml_dtypes

import concourse.bass as bass
import concourse.bacc as bacc
import concourse.mybir as mybir
from concourse import tile
from concourse.bass_utils import run_bass_kernel_spmd

B, H, W_, C = 8, 512, 512, 32
MODES = 32
N = 512
NCORES = 8

BF = mybir.dt.bfloat16
F32 = mybir.dt.float32


def _make_consts():
    h = np.arange(N)
    k = np.arange(MODES)
    ang = 2 * np.pi * np.outer(h, k) / N
    F = np.concatenate([np.cos(ang), -np.sin(ang)], axis=1)      # [512, 64]

    eye = np.eye(MODES)
    zc = np.concatenate([eye, np.zeros((MODES, N // 2 + 1 - MODES))], axis=1)
    row_re = np.fft.irfft(zc, n=N, axis=1)                        # [32, 512]
    row_im = np.fft.irfft(1j * zc, n=N, axis=1)

    # rows in interleaved (2*ky + ri) order, matching Tpc output rows
    Da = np.zeros((64, N))
    Db = np.zeros((64, N))
    Da[0::2] = row_re
    Da[1::2] = row_im
    Db[0::2] = row_im
    Db[1::2] = -row_re

    Einv = np.concatenate([np.cos(ang).T, np.sin(ang).T], axis=0) / N  # [64, 512]

    # F_sb[p, k*64+j] = F[k*128+p, j]
    F_sb = F.reshape(4, 128, 64).transpose(1, 0, 2).reshape(128, 256)
    Dab_sb = np.concatenate([Da, Db], axis=1)                          # [64, 1024]
    ident = np.eye(128)
    return (F_sb.astype(ml_dtypes.bfloat16), Dab_sb.astype(ml_dtypes.bfloat16),
            Einv.astype(ml_dtypes.bfloat16), ident.astype(np.float32),
            ident.astype(ml_dtypes.bfloat16))


def _build_nc():
    F_np, Dab_np, Einv_np, idf_np, idb_np = _make_consts()

    nc = bacc.Bacc()
    x_d = nc.dram_tensor("x", [H, W_ * C], BF, kind="ExternalInput")
    wpe_d = nc.dram_tensor("wpe", [32, 32768], BF, kind="ExternalInput")
    out_d = nc.dram_tensor("out", [H, W_ * C], BF, kind="ExternalOutput")

    f_c = nc.inline_tensor(F_np, name="f_const")
    dab_c = nc.inline_tensor(Dab_np, name="dab_const")
    einv_c = nc.inline_tensor(Einv_np, name="einv_const")
    idf_c = nc.inline_tensor(idf_np, name="idf_const")
    idb_c = nc.inline_tensor(idb_np, name="idb_const")

    with tile.TileContext(nc) as tc:
        with (
            tc.tile_pool(name="const", bufs=1) as cpool,
            tc.tile_pool(name="xp", bufs=5) as xpool,
            tc.tile_pool(name="mid", bufs=2) as midpool,
            tc.tile_pool(name="ptp", bufs=2) as ptpool,
            tc.tile_pool(name="wp", bufs=3) as wpool,
            tc.tile_pool(name="sml", bufs=1) as smlpool,
            tc.tile_pool(name="osb", bufs=4) as opool,
        ):
            # ---- constants to SBUF ----
            F_sb = cpool.tile([128, 256], BF)
            Dab_sb = cpool.tile([64, 1024], BF)
            Einv_sb = cpool.tile([64, 512], BF)
            ident_f = cpool.tile([128, 128], F32)
            ident_bf = cpool.tile([128, 128], BF)
            nc.sync.dma_start(ident_bf[:], idb_c[:])
            nc.sync.dma_start(F_sb[:], f_c[:])
            nc.sync.dma_start(ident_f[:], idf_c[:])
            nc.sync.dma_start(Dab_sb[:], dab_c[:])
            nc.sync.dma_start(Einv_sb[:], einv_c[:])

            rawT = smlpool.tile([32, 4096], F32)
            lowT = smlpool.tile([32, 2048], BF)
            OL_sb = smlpool.tile([32, 2048], BF)
            OLT = smlpool.tile([64, 1024], BF)
            uT = smlpool.tile([64, 16384], BF)
            raw_sb = smlpool.tile([64, 2048], F32)

            # PE warmup: release the HAM clock gate while the first
            # x tiles are still in flight (outputs never read)
            with tc.tile_pool(name="ps_w", bufs=1,
                              space=bass.MemorySpace.PSUM) as psw:
                wps = psw.tile([128, 512], F32, tag="wps", name="wps")
                for wi in range(48):
                    nc.tensor.matmul(
                        wps[:, 0:128], ident_bf[:], ident_bf[:],
                        start=True, stop=True)

            with (
                tc.tile_pool(name="ps_acc", bufs=4,
                             space=bass.MemorySpace.PSUM) as psa,
                tc.tile_pool(name="ps_pa", bufs=2,
                             space=bass.MemorySpace.PSUM) as ppa,
                tc.tile_pool(name="ps_pt1", bufs=2,
                             space=bass.MemorySpace.PSUM) as ppt1,
            ):
                # persistent stage-B accumulators
                pb = [psa.tile([64, 512], F32, tag="pb", bufs=4, name=f"pb{i}")
                      for i in range(4)]

                # ============= A + T1 + B, software-pipelined ============
                def emit_A(wq):
                    P_wq = midpool.tile([64, 4096], BF, tag="mid",
                                        name=f"P{wq}")
                    xk = []
                    for k in range(4):
                        t = xpool.tile([128, 4096], BF, tag="xk",
                                       name=f"x{wq}{k}")
                        nc.sync.dma_start(
                            t[:], x_d[k * 128:(k + 1) * 128,
                                      wq * 4096:(wq + 1) * 4096])
                        xk.append(t)
                    for ns in range(8):
                        pa = ppa.tile([64, 512], F32, tag="pa",
                                      name=f"pa{wq}{ns}")
                        for k in range(4):
                            nc.tensor.matmul(
                                pa[:], F_sb[:, k * 64:(k + 1) * 64],
                                xk[k][:, ns * 512:(ns + 1) * 512],
                                start=(k == 0), stop=(k == 3))
                        nc.scalar.copy(
                            P_wq[:, ns * 512:(ns + 1) * 512], pa[:])
                    return P_wq

                def emit_T1B(wq, P_wq):
                    PT_wq = ptpool.tile([128, 2048], BF, tag="pt",
                                        name=f"PT{wq}")
                    Pv = P_wq.rearrange("p (w c) -> p w c", c=32)
                    for cg in range(4):
                        pt1 = ppt1.tile([128, 512], BF, tag="pt1",
                                        name=f"pt1_{wq}{cg}")
                        for cl in range(8):
                            c = cg * 8 + cl
                            nc.tensor.transpose(
                                pt1[:, cl * 64:(cl + 1) * 64],
                                Pv[:, :, c], ident_bf[0:64, 0:64])
                        nc.vector.tensor_copy(
                            PT_wq[:, cg * 512:(cg + 1) * 512], pt1[:])
                    for ns in range(4):
                        nc.tensor.matmul(
                            pb[ns][:], F_sb[:, wq * 64:(wq + 1) * 64],
                            PT_wq[:, ns * 512:(ns + 1) * 512],
                            start=(wq == 0), stop=(wq == 3))

                P_prev = emit_A(0)
                for wq in range(1, 4):
                    P_cur = emit_A(wq)
                    emit_T1B(wq - 1, P_prev)
                    P_prev = P_cur
                emit_T1B(3, P_prev)

                for ns in range(4):
                    nc.vector.tensor_copy(
                        raw_sb[:, ns * 512:(ns + 1) * 512], pb[ns][:])

            # ============ Tc: c onto partitions, then re/im combine ======
            # rawT[p, kx*128 + rx*64 + ry*32 + ky] -> lowT[p, kx*64 + ry*32 + ky]
            rawv = raw_sb.rearrange("p (c k) -> p c k", k=64)
            rTv = rawT.rearrange("p (kx rx ry ky) -> p kx rx ry ky",
                                 kx=32, rx=2, ry=2)
            lTv4 = lowT.rearrange("p (kx ry ky) -> p kx ry ky", kx=32, ry=2)
            lTv = lowT.rearrange("p (kx ri ky) -> p kx ri ky", kx=32, ri=2)
            with (
                tc.tile_pool(name="ps_tc", bufs=2,
                             space=bass.MemorySpace.PSUM) as ptcp,
                tc.tile_pool(name="ps_c", bufs=4,
                             space=bass.MemorySpace.PSUM) as pcp,
                tc.tile_pool(name="ps_tpc", bufs=2,
                             space=bass.MemorySpace.PSUM) as ptpp,
            ):
                def emit_Tc(kxg):
                    ptc = ptcp.tile([32, 512], F32, tag="tc", name=f"ptc{kxg}")
                    for kxl in range(4):
                        kx = kxg * 4 + kxl
                        for rix in range(2):
                            nc.tensor.transpose(
                                ptc[:, kxl * 128 + rix * 64:
                                    kxl * 128 + rix * 64 + 64],
                                rawv[:, :, rix * 32 + kx], ident_f[0:64, 0:64])
                    nc.scalar.copy(rawT[:, kxg * 512:(kxg + 1) * 512], ptc[:])
                    k0, k1 = kxg * 4, kxg * 4 + 4
                    nc.vector.tensor_tensor(
                        lTv4[:, k0:k1, 0, :], rTv[:, k0:k1, 0, 0, :],
                        rTv[:, k0:k1, 1, 1, :], mybir.AluOpType.subtract)
                    nc.vector.tensor_tensor(
                        lTv4[:, k0:k1, 1, :], rTv[:, k0:k1, 0, 1, :],
                        rTv[:, k0:k1, 1, 0, :], mybir.AluOpType.add)

                def emit_C(g):
                    Wt = wpool.tile([32, 4096], BF, tag="wt", name=f"Wt{g}")
                    nc.sync.dma_start(Wt[:], wpe_d[:, g * 4096:(g + 1) * 4096])
                    pc = pcp.tile([32, 256], F32, tag="pc", name=f"pc{g}")
                    for kxl in range(4):
                        kx = g * 4 + kxl
                        for ky in range(32):
                            nc.tensor.matmul(
                                pc[:, kxl * 64 + ky * 2: kxl * 64 + ky * 2 + 2],
                                Wt[:, kxl * 1024 + ky * 32:
                                   kxl * 1024 + ky * 32 + 32],
                                lTv[:, kx, :, ky],
                                start=True, stop=True)
                    nc.scalar.copy(OL_sb[:, g * 256:(g + 1) * 256], pc[:])

                emit_Tc(0)
                for kxg in range(1, 8):
                    emit_Tc(kxg)
                    emit_C(kxg - 1)
                emit_C(7)

                # ============= Tpc: kyri onto partitions =================
                OLv = OL_sb.rearrange("p (kx q) -> p kx q", kx=32)
                for kxg in range(4):
                    ptp = ptpp.tile([64, 256], BF, tag="tpc", name=f"ptp{kxg}")
                    for kxl in range(8):
                        kx = kxg * 8 + kxl
                        nc.tensor.transpose(
                            ptp[:, kxl * 32:(kxl + 1) * 32],
                            OLv[:, kx, :], ident_bf[0:32, 0:32])
                    nc.scalar.copy(OLT[:, kxg * 256:(kxg + 1) * 256], ptp[:])

            # ================= D + T2 per w-chunk ========================
            uTv = uT.rearrange("p (w d) -> p w d", d=32)
            with (
                tc.tile_pool(name="ps_d", bufs=3,
                             space=bass.MemorySpace.PSUM) as pdp,
                tc.tile_pool(name="ps_t2", bufs=3,
                             space=bass.MemorySpace.PSUM) as pt2p,
            ):
                for wc in range(4):
                    u_wc = midpool.tile([128, 2048], BF, tag="mid",
                                        name=f"u{wc}")
                    for ab in range(2):
                        for ns in range(2):
                            pd = pdp.tile([128, 512], F32, tag="pd",
                                          name=f"pd{wc}{ab}{ns}")
                            nc.tensor.matmul(
                                pd[:],
                                Dab_sb[:, ab * 512 + wc * 128:
                                       ab * 512 + (wc + 1) * 128],
                                OLT[:, ns * 512:(ns + 1) * 512],
                                start=True, stop=True)
                            eng = nc.scalar if ns == 0 else nc.vector
                            if ns == 0:
                                nc.scalar.copy(
                                    u_wc[:, ab * 1024 + ns * 512:
                                         ab * 1024 + (ns + 1) * 512],
                                    pd[:])
                            else:
                                nc.vector.tensor_copy(
                                    u_wc[:, ab * 1024 + ns * 512:
                                         ab * 1024 + (ns + 1) * 512],
                                    pd[:])
                    uv = u_wc.rearrange("p (ab kx d) -> p ab kx d", ab=2, d=32)
                    for dg in range(8):
                        pt2 = pt2p.tile([64, 512], BF, tag="pt2",
                                        name=f"pt2_{wc}{dg}")
                        for dl in range(4):
                            d = dg * 4 + dl
                            nc.tensor.transpose(
                                pt2[:, dl * 128:(dl + 1) * 128],
                                uv[:, :, :, d], ident_bf[:])
                        # one copy: dest scan (w, dl) <- src cols dl*128 + w
                        p2v = pt2.rearrange("p (dl w) -> p w dl", w=128)
                        if dg % 2 == 0:
                            nc.vector.tensor_copy(
                                uTv[:, wc * 128:(wc + 1) * 128,
                                    dg * 4:dg * 4 + 4],
                                p2v[:])
                        else:
                            nc.scalar.copy(
                                uTv[:, wc * 128:(wc + 1) * 128,
                                    dg * 4:dg * 4 + 4],
                                p2v[:])

            # ================= E + DMA out ===============================
            with tc.tile_pool(name="ps_e", bufs=6,
                              space=bass.MemorySpace.PSUM) as pse:
                for qb in range(4):
                    for hc in range(4):
                        ob = opool.tile([128, 4096], BF, tag="osb",
                                        name=f"ob{hc}{qb}")
                        for si in range(8):
                            nb = qb * 8 + si
                            pe_t = pse.tile([128, 512], F32, tag="pse",
                                            name=f"pe{hc}{qb}{si}")
                            nc.tensor.matmul(
                                pe_t[:],
                                Einv_sb[:, hc * 128:(hc + 1) * 128],
                                uT[:, nb * 512:(nb + 1) * 512],
                                start=True, stop=True)
                            if (hc * 4 + qb) % 2 == 0:
                                nc.vector.tensor_copy(
                                    ob[:, si * 512:(si + 1) * 512], pe_t[:])
                            else:
                                nc.scalar.copy(
                                    ob[:, si * 512:(si + 1) * 512], pe_t[:])
                        nc.sync.dma_start(
                            out_d[hc * 128:(hc + 1) * 128,
                                  qb * 4096:(qb + 1) * 4096],
                            ob[:])
    nc.compile()
    return nc


_NC_CACHE = {}


def _get_nc():
    if "nc" not in _NC_CACHE:
        _NC_CACHE["nc"] = _build_nc()
    return _NC_CACHE["nc"]


def _wpe_from_W(W):
    # wpe[c, kx*1024 + ky*32 + d] = W[kx, ky, c, d]
    Wt = np.asarray(W, dtype=np.float32)
    wpe = Wt.transpose(2, 0, 1, 3).reshape(32, 32768)
    return np.ascontiguousarray(wpe.astype(ml_dtypes.bfloat16))


def kernel(x, W):
    xb = np.asarray(x).reshape(NCORES, H, W_ * C).astype(ml_dtypes.bfloat16)
    wpe = _wpe_from_W(W)
    nc = _get_nc()
    in_maps = [{"x": np.ascontiguousarray(xb[i]), "wpe": wpe}
               for i in range(NCORES)]
    res = run_bass_kernel_spmd(nc, in_maps, list(range(NCORES))).results
    out = np.stack([res[i]["out"].reshape(H, W_, C) for i in range(NCORES)])
    return out.astype(np.float32)


if __name__ == "__main__":
    rng = np.random.default_rng(0)
    x = rng.standard_normal((B, H, W_, C)).astype(np.float32)
    W = rng.standard_normal((MODES, MODES, C, C)).astype(np.float32) * 0.125
    out = kernel(x, W)
    print(out.shape, out.dtype)


# revision 17
# speedup vs baseline: 1.1391x; 1.1391x over previous
"""Trainium2 Bass kernel for the FNO-style FourierLayer.

  x: [8, 512, 512, 32] f32 -> rfft2 over (h, w) -> keep 32x32 modes ->
  per-mode (C x C) channel mix with W[32, 32, 32, 32] -> zero-pad -> irfft2.

Strategy: data-parallel over batch, one sample per NeuronCore (8 cores).
Only 32 of 512 frequencies survive, so instead of an FFT each core runs a
chain of small dense real matmuls against DFT basis matrices (bf16 operands,
fp32 PSUM accumulation):

  A:   P = F^T X         contract h       -> [kxri 64, (w c)]
  T1:  PE transposes     w onto partitions
  B:   raw = G^T PT      contract w       -> [kyri 64, (c kxri)] psum accum
  Tc:  PE transposes     c onto partitions -> rawT [32, (kx rx kyri)]
  CMB: complex re/im combine (DVE, column-wise) -> lowT [32, (kx ry ky)]
  C:   per-mode matmuls  OL = W^T low     -> [d 32, (kx ky ri)]
  Tpc: PE transposes     kyri onto partitions -> OLT [64, (kx d)]
  D:   u = Dab^T OLT     contract kyri    -> u_wc [w 128, (ab kx d)]
  T2:  PE transposes     (ab kx) onto partitions -> uT [64, (w d)]
  E:   out = Einv^T uT   contract (ab kx) -> [h 128, (w d)] -> DMA out

DFT matrices are built on host from np.fft basis responses (this captures
the irfft Im(DC)-drop convention exactly). x, W and the matrices are cast
to bf16 on host and the output is returned as bf16 and upcast on host,
which halves DMA traffic in both directions.
"""
import numpy as np
import ml_dtypes

import concourse.bass as bass
import concourse.bacc as bacc
import concourse.mybir as mybir
from concourse import tile
from concourse.bass_utils import run_bass_kernel_spmd

B, H, W_, C = 8, 512, 512, 32
MODES = 32
N = 512
NCORES = 8

BF = mybir.dt.bfloat16
F32 = mybir.dt.float32


def _make_consts():
    h = np.arange(N)
    k = np.arange(MODES)
    ang = 2 * np.pi * np.outer(h, k) / N
    F = np.concatenate([np.cos(ang), -np.sin(ang)], axis=1)      # [512, 64]

    eye = np.eye(MODES)
    zc = np.concatenate([eye, np.zeros((MODES, N // 2 + 1 - MODES))], axis=1)
    row_re = np.fft.irfft(zc, n=N, axis=1)                        # [32, 512]
    row_im = np.fft.irfft(1j * zc, n=N, axis=1)

    # rows in interleaved (2*ky + ri) order, matching Tpc output rows
    Da = np.zeros((64, N))
    Db = np.zeros((64, N))
    Da[0::2] = row_re
    Da[1::2] = row_im
    Db[0::2] = row_im
    Db[1::2] = -row_re

    Einv = np.concatenate([np.cos(ang).T, np.sin(ang).T], axis=0) / N  # [64, 512]

    # F_sb[p, k*64+j] = F[k*128+p, j]
    F_sb = F.reshape(4, 128, 64).transpose(1, 0, 2).reshape(128, 256)
    Dab_sb = np.concatenate([Da, Db], axis=1)                          # [64, 1024]
    ident = np.eye(128)
    return (F_sb.astype(ml_dtypes.bfloat16), Dab_sb.astype(ml_dtypes.bfloat16),
            Einv.astype(ml_dtypes.bfloat16), ident.astype(np.float32),
            ident.astype(ml_dtypes.bfloat16))


def _build_nc():
    F_np, Dab_np, Einv_np, idf_np, idb_np = _make_consts()

    nc = bacc.Bacc()
    x_d = nc.dram_tensor("x", [H, W_ * C], BF, kind="ExternalInput")
    wpe_d = nc.dram_tensor("wpe", [32, 32768], BF, kind="ExternalInput")
    out_d = nc.dram_tensor("out", [H, W_ * C], BF, kind="ExternalOutput")

    f_c = nc.inline_tensor(F_np, name="f_const")
    dab_c = nc.inline_tensor(Dab_np, name="dab_const")
    einv_c = nc.inline_tensor(Einv_np, name="einv_const")
    idf_c = nc.inline_tensor(idf_np, name="idf_const")
    idb_c = nc.inline_tensor(idb_np, name="idb_const")

    with tile.TileContext(nc) as tc:
        with (
            tc.tile_pool(name="const", bufs=1) as cpool,
            tc.tile_pool(name="xp", bufs=6) as xpool,
            tc.tile_pool(name="mid", bufs=2) as midpool,
            tc.tile_pool(name="ptp", bufs=2) as ptpool,
            tc.tile_pool(name="wp", bufs=3) as wpool,
            tc.tile_pool(name="sml", bufs=1) as smlpool,
            tc.tile_pool(name="osb", bufs=3) as opool,
        ):
            # ---- constants to SBUF ----
            F_sb = cpool.tile([128, 256], BF)
            Dab_sb = cpool.tile([64, 1024], BF)
            Einv_sb = cpool.tile([64, 512], BF)
            ident_f = cpool.tile([128, 128], F32)
            ident_bf = cpool.tile([128, 128], BF)
            nc.sync.dma_start(ident_bf[:], idb_c[:])
            nc.sync.dma_start(F_sb[:], f_c[:])
            nc.sync.dma_start(ident_f[:], idf_c[:])
            nc.sync.dma_start(Dab_sb[:], dab_c[:])
            nc.sync.dma_start(Einv_sb[:], einv_c[:])

            rawT = smlpool.tile([32, 4096], F32, tag="bigshare", name="rawT")
            lowT = smlpool.tile([32, 2048], BF)
            OL_sb = smlpool.tile([32, 2048], BF)
            OLT = smlpool.tile([64, 1024], BF)
            uT = smlpool.tile([64, 16384], BF, tag="bigshare", name="uT")
            raw_sb = smlpool.tile([64, 2048], F32)

            # PE warmup: release the HAM clock gate while the first
            # x tiles are still in flight (outputs never read)
            with tc.tile_pool(name="ps_w", bufs=1,
                              space=bass.MemorySpace.PSUM) as psw:
                wps = psw.tile([128, 512], F32, tag="wps", name="wps")
                for wi in range(48):
                    nc.tensor.matmul(
                        wps[:, 0:128], ident_bf[:], ident_bf[:],
                        start=True, stop=True)

            with (
                tc.tile_pool(name="ps_acc", bufs=4,
                             space=bass.MemorySpace.PSUM) as psa,
                tc.tile_pool(name="ps_pa", bufs=2,
                             space=bass.MemorySpace.PSUM) as ppa,
                tc.tile_pool(name="ps_pt1", bufs=2,
                             space=bass.MemorySpace.PSUM) as ppt1,
            ):
                # persistent stage-B accumulators
                pb = [psa.tile([64, 512], F32, tag="pb", bufs=4, name=f"pb{i}")
                      for i in range(4)]

                # ============= A + T1 + B, software-pipelined ============
                def emit_A(wq):
                    P_wq = midpool.tile([64, 4096], BF, tag="mid",
                                        name=f"P{wq}")
                    xk = []
                    for k in range(4):
                        t = xpool.tile([128, 4096], BF, tag="xk",
                                       name=f"x{wq}{k}")
                        nc.sync.dma_start(
                            t[:], x_d[k * 128:(k + 1) * 128,
                                      wq * 4096:(wq + 1) * 4096])
                        xk.append(t)
                    for ns in range(8):
                        pa = ppa.tile([64, 512], F32, tag="pa",
                                      name=f"pa{wq}{ns}")
                        for k in range(4):
                            nc.tensor.matmul(
                                pa[:], F_sb[:, k * 64:(k + 1) * 64],
                                xk[k][:, ns * 512:(ns + 1) * 512],
                                start=(k == 0), stop=(k == 3))
                        nc.scalar.copy(
                            P_wq[:, ns * 512:(ns + 1) * 512], pa[:])
                    return P_wq

                def emit_T1B(wq, P_wq):
                    PT_wq = ptpool.tile([128, 2048], BF, tag="pt",
                                        name=f"PT{wq}")
                    Pv = P_wq.rearrange("p (w c) -> p w c", c=32)
                    for cg in range(4):
                        pt1 = ppt1.tile([128, 512], BF, tag="pt1",
                                        name=f"pt1_{wq}{cg}")
                        for cl in range(8):
                            c = cg * 8 + cl
                            nc.tensor.transpose(
                                pt1[:, cl * 64:(cl + 1) * 64],
                                Pv[:, :, c], ident_bf[0:64, 0:64])
                        nc.vector.tensor_copy(
                            PT_wq[:, cg * 512:(cg + 1) * 512], pt1[:])
                    for ns in range(4):
                        nc.tensor.matmul(
                            pb[ns][:], F_sb[:, wq * 64:(wq + 1) * 64],
                            PT_wq[:, ns * 512:(ns + 1) * 512],
                            start=(wq == 0), stop=(wq == 3))

                P_prev = emit_A(0)
                for wq in range(1, 4):
                    P_cur = emit_A(wq)
                    emit_T1B(wq - 1, P_prev)
                    P_prev = P_cur
                emit_T1B(3, P_prev)

                for ns in range(4):
                    nc.vector.tensor_copy(
                        raw_sb[:, ns * 512:(ns + 1) * 512], pb[ns][:])

            # ============ Tc: c onto partitions, then re/im combine ======
            # rawT[p, kx*128 + rx*64 + ry*32 + ky] -> lowT[p, kx*64 + ry*32 + ky]
            rawv = raw_sb.rearrange("p (c k) -> p c k", k=64)
            rTv = rawT.rearrange("p (kx rx ry ky) -> p kx rx ry ky",
                                 kx=32, rx=2, ry=2)
            lTv4 = lowT.rearrange("p (kx ry ky) -> p kx ry ky", kx=32, ry=2)
            lTv = lowT.rearrange("p (kx ri ky) -> p kx ri ky", kx=32, ri=2)
            with (
                tc.tile_pool(name="ps_tc", bufs=2,
                             space=bass.MemorySpace.PSUM) as ptcp,
                tc.tile_pool(name="ps_c", bufs=4,
                             space=bass.MemorySpace.PSUM) as pcp,
                tc.tile_pool(name="ps_tpc", bufs=2,
                             space=bass.MemorySpace.PSUM) as ptpp,
            ):
                def emit_Tc(kxg):
                    ptc = ptcp.tile([32, 512], F32, tag="tc", name=f"ptc{kxg}")
                    for kxl in range(4):
                        kx = kxg * 4 + kxl
                        for rix in range(2):
                            nc.tensor.transpose(
                                ptc[:, kxl * 128 + rix * 64:
                                    kxl * 128 + rix * 64 + 64],
                                rawv[:, :, rix * 32 + kx], ident_f[0:64, 0:64])
                    nc.scalar.copy(rawT[:, kxg * 512:(kxg + 1) * 512], ptc[:])
                    k0, k1 = kxg * 4, kxg * 4 + 4
                    nc.vector.tensor_tensor(
                        lTv4[:, k0:k1, 0, :], rTv[:, k0:k1, 0, 0, :],
                        rTv[:, k0:k1, 1, 1, :], mybir.AluOpType.subtract)
                    nc.vector.tensor_tensor(
                        lTv4[:, k0:k1, 1, :], rTv[:, k0:k1, 0, 1, :],
                        rTv[:, k0:k1, 1, 0, :], mybir.AluOpType.add)

                def emit_C(g):
                    Wt = wpool.tile([32, 4096], BF, tag="wt", name=f"Wt{g}")
                    nc.sync.dma_start(Wt[:], wpe_d[:, g * 4096:(g + 1) * 4096])
                    pc = pcp.tile([32, 256], F32, tag="pc", name=f"pc{g}")
                    for kxl in range(4):
                        kx = g * 4 + kxl
                        for ky in range(32):
                            nc.tensor.matmul(
                                pc[:, kxl * 64 + ky * 2: kxl * 64 + ky * 2 + 2],
                                Wt[:, kxl * 1024 + ky * 32:
                                   kxl * 1024 + ky * 32 + 32],
                                lTv[:, kx, :, ky],
                                start=True, stop=True)
                    nc.scalar.copy(OL_sb[:, g * 256:(g + 1) * 256], pc[:])

                emit_Tc(0)
                for kxg in range(1, 8):
                    emit_Tc(kxg)
                    emit_C(kxg - 1)
                emit_C(7)

                # ============= Tpc: kyri onto partitions =================
                OLv = OL_sb.rearrange("p (kx q) -> p kx q", kx=32)
                for kxg in range(4):
                    ptp = ptpp.tile([64, 256], BF, tag="tpc", name=f"ptp{kxg}")
                    for kxl in range(8):
                        kx = kxg * 8 + kxl
                        nc.tensor.transpose(
                            ptp[:, kxl * 32:(kxl + 1) * 32],
                            OLv[:, kx, :], ident_bf[0:32, 0:32])
                    nc.scalar.copy(OLT[:, kxg * 256:(kxg + 1) * 256], ptp[:])

            # ================= D + T2 per w-chunk ========================
            uTv = uT.rearrange("p (w d) -> p w d", d=32)
            with (
                tc.tile_pool(name="ps_d", bufs=3,
                             space=bass.MemorySpace.PSUM) as pdp,
                tc.tile_pool(name="ps_t2", bufs=3,
                             space=bass.MemorySpace.PSUM) as pt2p,
            ):
                for wc in range(4):
                    u_wc = midpool.tile([128, 2048], BF, tag="mid",
                                        name=f"u{wc}")
                    for ab in range(2):
                        for ns in range(2):
                            pd = pdp.tile([128, 512], F32, tag="pd",
                                          name=f"pd{wc}{ab}{ns}")
                            nc.tensor.matmul(
                                pd[:],
                                Dab_sb[:, ab * 512 + wc * 128:
                                       ab * 512 + (wc + 1) * 128],
                                OLT[:, ns * 512:(ns + 1) * 512],
                                start=True, stop=True)
                            eng = nc.scalar if ns == 0 else nc.vector
                            if ns == 0:
                                nc.scalar.copy(
                                    u_wc[:, ab * 1024 + ns * 512:
                                         ab * 1024 + (ns + 1) * 512],
                                    pd[:])
                            else:
                                nc.vector.tensor_copy(
                                    u_wc[:, ab * 1024 + ns * 512:
                                         ab * 1024 + (ns + 1) * 512],
                                    pd[:])
                    uv = u_wc.rearrange("p (ab kx d) -> p ab kx d", ab=2, d=32)
                    for dg in range(8):
                        pt2 = pt2p.tile([64, 512], BF, tag="pt2",
                                        name=f"pt2_{wc}{dg}")
                        for dl in range(4):
                            d = dg * 4 + dl
                            nc.tensor.transpose(
                                pt2[:, dl * 128:(dl + 1) * 128],
                                uv[:, :, :, d], ident_bf[:])
                        # one copy: dest scan (w, dl) <- src cols dl*128 + w
                        p2v = pt2.rearrange("p (dl w) -> p w dl", w=128)
                        if dg % 2 == 0:
                            nc.vector.tensor_copy(
                                uTv[:, wc * 128:(wc + 1) * 128,
                                    dg * 4:dg * 4 + 4],
                                p2v[:])
                        else:
                            nc.scalar.copy(
                                uTv[:, wc * 128:(wc + 1) * 128,
                                    dg * 4:dg * 4 + 4],
                                p2v[:])

            # ================= E + DMA out ===============================
            with tc.tile_pool(name="ps_e", bufs=6,
                              space=bass.MemorySpace.PSUM) as pse:
                for qb in range(4):
                    for hc in range(4):
                        ob = opool.tile([128, 4096], BF, tag="osb",
                                        name=f"ob{hc}{qb}")
                        for si in range(8):
                            nb = qb * 8 + si
                            pe_t = pse.tile([128, 512], F32, tag="pse",
                                            name=f"pe{hc}{qb}{si}")
                            nc.tensor.matmul(
                                pe_t[:],
                                Einv_sb[:, hc * 128:(hc + 1) * 128],
                                uT[:, nb * 512:(nb + 1) * 512],
                                start=True, stop=True)
                            if si % 2 == 0:
                                nc.vector.tensor_copy(
                                    ob[:, si * 512:(si + 1) * 512], pe_t[:])
                            else:
                                nc.scalar.copy(
                                    ob[:, si * 512:(si + 1) * 512], pe_t[:])
                        nc.sync.dma_start(
                            out_d[hc * 128:(hc + 1) * 128,
                                  qb * 4096:(qb + 1) * 4096],
                            ob[:])
    nc.compile()
    return nc


_NC_CACHE = {}


def _get_nc():
    if "nc" not in _NC_CACHE:
        _NC_CACHE["nc"] = _build_nc()
    return _NC_CACHE["nc"]


def _wpe_from_W(W):
    # wpe[c, kx*1024 + ky*32 + d] = W[kx, ky, c, d]
    Wt = np.asarray(W, dtype=np.float32)
    wpe = Wt.transpose(2, 0, 1, 3).reshape(32, 32768)
    return np.ascontiguousarray(wpe.astype(ml_dtypes.bfloat16))


def kernel(x, W):
    xb = np.asarray(x).reshape(NCORES, H, W_ * C).astype(ml_dtypes.bfloat16)
    wpe = _wpe_from_W(W)
    nc = _get_nc()
    in_maps = [{"x": np.ascontiguousarray(xb[i]), "wpe": wpe}
               for i in range(NCORES)]
    res = run_bass_kernel_spmd(nc, in_maps, list(range(NCORES))).results
    out = np.stack([res[i]["out"].reshape(H, W_, C) for i in range(NCORES)])
    return out.astype(np.float32)


if __name__ == "__main__":
    rng = np.random.default_rng(0)
    x = rng.standard_normal((B, H, W_, C)).astype(np.float32)
    W = rng.standard_normal((MODES, MODES, C, C)).astype(np.float32) * 0.125
    out = kernel(x, W)
    print(out.shape, out.dtype)
